# revision 45
# baseline (speedup 1.0000x reference)
"""Multi-head causal attention (B=2, S=2048, D=4096, H=32, hd=128) on 8 trn2 cores.

Sharding: DP over batch (2) x TP over heads (4 groups of 8 heads).
Core c: batch b = c//4, head-group tp = c%4.
Each core computes a partial output [2048, 4096] (wo row-sharded); host sums
the 4 partials per batch (bf16 partials, f32 accumulate).

Design (vs the f32r spill-based baseline; ~13% faster per-core exec):
- bf16 on the whole PE datapath (same 1 cycle/row as f32r, half the DMA);
  PSUM accumulation stays f32.
- q/k/vT are RESIDENT in SBUF between phases (no DRAM spill round-trip).
- v is produced directly transposed ([tok, feat]) by swapping matmul operands,
  so phase 2 needs no PE transposes / DVE copies of v.
- RoPE runs on full 128-partition tiles with host-stacked cos/sin tables
  (half the DVE instructions of the 64-row variant).
- causal masking uses ONE resident [128,512] boundary pattern; boundary
  blocks compute scores/exp/PV/denominator only on the unmasked free slice.
- phase 2 is a single software pipeline across all 32 (head, strip) chains
  (scores run DEPTH=4 blocks ahead of exp/PV) so the exp latency is exposed
  once, not per chain; phase 2 runs PE and Act engines ~fully co-saturated.
- weights are host-pre-tiled so every DMA is 1 descriptor per partition;
  wo chunk 0 prefetches during phase 2.
"""

import sys
sys.path.insert(0, '/opt/trn_rl_repo')
sys.path.insert(0, '/opt/trn_rl_repo/concourse')

import numpy as np

S = 2048
D = 4096
HD = 128
FSH = 1024            # features per core (8 heads)
NHL = 8               # heads per core
KT = D // 128         # 32 contraction tiles
NST = 4               # token strips of 512 (phase 1 and phase 2)
NKT = S // 128        # 16 tk blocks
NEG_THRESH = -1.0e8
NEG = -1.0e9

_cache = {}


def _build(classes, n3):
    """classes[j][s] in {0:skip, 1:plain, 2:boundary r=j*128-s*512, 3:extra}.
    n3 = number of packed extra full-mask blocks (class 3)."""
    import concourse.bacc as bacc
    import concourse.mybir as mybir
    import concourse.tile as tile

    f32 = mybir.dt.float32
    bf16 = mybir.dt.bfloat16
    EXP = mybir.ActivationFunctionType.Exp
    COPY = mybir.ActivationFunctionType.Copy

    nc = bacc.Bacc("TRN2", target_bir_lowering=False, debug=False)

    xg_d = nc.dram_tensor("xg", [NST * 128, KT * 512], bf16, kind="ExternalInput").ap()
    wqg_d = nc.dram_tensor("wqg", [NHL * 128, KT * 128], bf16, kind="ExternalInput").ap()
    wkg_d = nc.dram_tensor("wkg", [NHL * 128, KT * 128], bf16, kind="ExternalInput").ap()
    wvg_d = nc.dram_tensor("wvg", [8 * 128, 8 * 512], bf16, kind="ExternalInput").ap()
    wog_d = nc.dram_tensor("wog", [8 * 128, NHL * 512], bf16, kind="ExternalInput").ap()
    cos2_d = nc.dram_tensor("cos2", [128, S], bf16, kind="ExternalInput").ap()
    sin2_d = nc.dram_tensor("sin2", [128, S], bf16, kind="ExternalInput").ap()
    m0_d = nc.dram_tensor("m0", [128, 512], bf16, kind="ExternalInput").ap()
    ones_d = nc.dram_tensor("ones8", [128, 8], bf16, kind="ExternalInput").ap()
    x3_d = nc.dram_tensor("x3", [128, max(1, n3) * 512], f32, kind="ExternalInput").ap()
    out_d = nc.dram_tensor("out", [S, D], bf16, kind="ExternalOutput").ap()

    # per-strip active-block schedule (first block must be full-width)
    sched = []
    for s in range(NST):
        full = [j for j in range(NKT) if classes[j][s] == 1]
        xtra = [j for j in range(NKT) if classes[j][s] == 3]
        bnd = sorted((j for j in range(NKT) if classes[j][s] == 2),
                     key=lambda j: j * 128 - s * 512)
        seq = full + xtra + bnd
        assert seq, f"strip {s} has no active blocks"
        j0 = seq[0]
        assert classes[j0][s] in (1, 3) or j0 * 128 - s * 512 <= 0, \
            f"strip {s} first block {j0} is not full-width"
        sched.append(seq)

    idx3 = {}
    for j in range(NKT):
        for s in range(NST):
            if classes[j][s] == 3:
                idx3[(j, s)] = len(idx3)

    with tile.TileContext(nc) as tc, \
         nc.allow_low_precision(reason="bf16 datapath, f32 accumulation"):
        with tc.tile_pool(name="pconst", bufs=1) as pconst, \
             tc.tile_pool(name="pqkv", bufs=1) as pqkv:
            m0_sb = pconst.tile([128, 512], bf16, name="m0_sb")
            nc.scalar.dma_start(out=m0_sb, in_=m0_d)
            ones_sb = pconst.tile([128, 8], bf16, name="ones_sb")
            nc.scalar.dma_start(out=ones_sb, in_=ones_d)
            if n3:
                x3_sb = pconst.tile([128, n3, 512], f32, name="x3_sb")
                nc.scalar.dma_start(out=x3_sb,
                                    in_=x3_d.rearrange("p (n f) -> p n f", n=n3))

            q_sb = [pqkv.tile([128, S], bf16, name=f"q{i}") for i in range(NHL)]
            k_sb = [pqkv.tile([128, S], bf16, name=f"k{i}") for i in range(NHL)]
            vT_sb = [pqkv.tile([128, FSH], bf16, name=f"vT{t}") for t in range(NKT)]

            # ---------------- Phase 1: projections (+RoPE on q,k) ----------
            with tc.tile_pool(name="px", bufs=3) as px, \
                 tc.tile_pool(name="pw", bufs=3) as pw, \
                 tc.tile_pool(name="pwv", bufs=4) as pwv, \
                 tc.tile_pool(name="ptab", bufs=2) as ptab, \
                 tc.tile_pool(name="prope", bufs=2) as prope, \
                 tc.tile_pool(name="pp1", bufs=4, space="PSUM") as pp1:
                for st in range(NST):
                    t0 = st * 512
                    xa = px.tile([128, 16, 512], bf16, name="xk")
                    if st == 0:
                        # split the first load so the PE can start on the
                        # first 8 k-tiles while the rest streams in
                        nc.sync.dma_start(
                            out=xa[:, 0:8, :],
                            in_=xg_d[0:128, 0:4096]
                            .rearrange("p (k f) -> p k f", k=8))
                        nc.sync.dma_start(
                            out=xa[:, 8:16, :],
                            in_=xg_d[0:128, 4096:8192]
                            .rearrange("p (k f) -> p k f", k=8))
                    else:
                        nc.sync.dma_start(
                            out=xa, in_=xg_d[st * 128:(st + 1) * 128, 0:8192]
                            .rearrange("p (k f) -> p k f", k=16))
                    wt0 = None
                    if st == 0:
                        wt0 = pw.tile([128, KT, 128], bf16, name="wt")
                        nc.scalar.dma_start(
                            out=wt0, in_=wqg_d[0:128, :]
                            .rearrange("p (k f) -> p k f", k=KT))
                    xb = px.tile([128, 16, 512], bf16, name="xk")
                    nc.sync.dma_start(
                        out=xb, in_=xg_d[st * 128:(st + 1) * 128, 8192:16384]
                        .rearrange("p (k f) -> p k f", k=16))
                    cos_t = ptab.tile([128, 512], bf16, name="tab")
                    nc.sync.dma_start(out=cos_t, in_=cos2_d[:, t0:t0 + 512])
                    sin_t = ptab.tile([128, 512], bf16, name="tab")
                    nc.sync.dma_start(out=sin_t, in_=sin2_d[:, t0:t0 + 512])

                    def xk(k):
                        return (xa if k < 16 else xb)[:, k % 16, :]

                    for wg_d, dst_list in ((wqg_d, q_sb), (wkg_d, k_sb)):
                        for i in range(NHL):
                            if wt0 is not None:
                                wt, wt0 = wt0, None
                            else:
                                wt = pw.tile([128, KT, 128], bf16, name="wt")
                                nc.scalar.dma_start(
                                    out=wt, in_=wg_d[i * 128:(i + 1) * 128, :]
                                    .rearrange("p (k f) -> p k f", k=KT))
                            ps = pp1.tile([128, 512], f32, name="ps1")
                            for k in range(KT):
                                nc.tensor.matmul(ps, wt[:, k, :], xk(k),
                                                 start=(k == 0), stop=(k == KT - 1))
                            m_c = prope.tile([128, 512], f32, name="rope")
                            nc.vector.tensor_mul(m_c, ps, cos_t)
                            m_s = prope.tile([128, 512], f32, name="rope")
                            nc.vector.tensor_mul(m_s[0:64], ps[64:128], sin_t[0:64])
                            nc.vector.tensor_mul(m_s[64:128], ps[0:64], sin_t[64:128])
                            nc.vector.tensor_add(dst_list[i][:, t0:t0 + 512], m_c, m_s)

                    for fc in range(2):
                        # two half-passes over k so only 2 of 4 weight
                        # quarter-groups are live at once
                        psv = [pp1.tile([128, 512], f32, name="psv")
                               for _ in range(4)]
                        for half in range(2):
                            wvt = []
                            for g in range(2):
                                wvq = pwv.tile([128, 8, 512], bf16, name="wvq")
                                row0 = (fc * 4 + half * 2 + g) * 128
                                nc.sync.dma_start(
                                    out=wvq, in_=wvg_d[row0:row0 + 128, :]
                                    .rearrange("p (k f) -> p k f", k=8))
                                wvt.append(wvq)
                            for tt in range(4):
                                for kk in range(16):
                                    k = half * 16 + kk
                                    wv_t = wvt[kk // 8][:, kk % 8, :]
                                    nc.tensor.matmul(
                                        psv[tt],
                                        xk(k)[:, tt * 128:(tt + 1) * 128], wv_t,
                                        start=(k == 0), stop=(k == KT - 1),
                                        skip_group_check=True)
                        for tt in range(4):
                            nc.scalar.activation(
                                vT_sb[st * 4 + tt][:, fc * 512:(fc + 1) * 512],
                                psv[tt], COPY)

            # ---------------- Phases 2+3 ----------------------------------
            with tc.tile_pool(name="patt", bufs=1) as patt, \
                 tc.tile_pool(name="pw3", bufs=2) as pw3, \
                 tc.tile_pool(name="po3", bufs=4) as po3:
                att_sb = [patt.tile([128, S], bf16, name=f"att{h}")
                          for h in range(NHL)]
                w3_tiles = [None] * 8

                def load_w3(c):
                    wt = pw3.tile([128, NHL, 512], bf16, name="w3")
                    nc.sync.dma_start(
                        out=wt, in_=wog_d[c * 128:(c + 1) * 128, :]
                        .rearrange("p (k f) -> p k f", k=NHL))
                    return wt

                w3_tiles[0] = load_w3(0)   # prefetch during phase 2

                # ---------------- Phase 2: attention per head -------------
                with tc.tile_pool(name="pE", bufs=6) as pE, \
                     tc.tile_pool(name="pms", bufs=4) as pms, \
                     tc.tile_pool(name="prec", bufs=2) as prec, \
                     tc.tile_pool(name="pbsb", bufs=2) as pbsb, \
                     tc.tile_pool(name="pss", bufs=4, space="PSUM") as pss, \
                     tc.tile_pool(name="psa", bufs=2, space="PSUM") as psa, \
                     tc.tile_pool(name="psd", bufs=2, space="PSUM") as psd:
                    # one software pipeline across ALL (head, strip) chains:
                    # scores run DEPTH blocks ahead of exp/PV/denominator so
                    # the exp latency is only exposed once, not per chain
                    from collections import deque

                    def emit_scores(ch, idx):
                        h, s, seq, A, Dn, sps = ch
                        j = seq[idx]
                        sp = pss.tile([128, 512], f32, name="sps")
                        nc.tensor.matmul(
                            sp, k_sb[h][:, j * 128:(j + 1) * 128],
                            q_sb[h][:, s * 512:(s + 1) * 512],
                            start=True, stop=True)
                        sps[idx] = sp

                    def emit_avdn(ch, idx):
                        h, s, seq, A, Dn, sps = ch
                        n = len(seq)
                        j = seq[idx]
                        cls = classes[j][s]
                        sp = sps[idx]
                        sps[idx] = None
                        r = max(j * 128 - s * 512, 0) if cls == 2 else 0
                        w = 512 - r
                        if cls == 1:
                            E = pE.tile([128, 512], bf16, name="E")
                            nc.scalar.activation(E, sp, EXP)
                            rhs = E
                        elif cls == 3:
                            ms = pms.tile([128, 512], f32, name="ms")
                            nc.vector.tensor_add(
                                ms, sp, x3_sb[:, idx3[(j, s)], :])
                            E = pE.tile([128, 512], bf16, name="E")
                            nc.scalar.activation(E, ms, EXP)
                            rhs = E
                        else:
                            ms = pms.tile([128, 512], f32, name="ms")
                            nc.vector.tensor_add(
                                ms[:, 0:w], sp[:, r:512], m0_sb[:, 0:w])
                            E = pE.tile([128, 512], bf16, name="E")
                            nc.scalar.activation(E[:, 0:w], ms[:, 0:w], EXP)
                            rhs = E[:, 0:w]
                        first, last = (idx == 0), (idx == n - 1)
                        nc.tensor.matmul(
                            A[:, r:512] if r else A,
                            vT_sb[j][:, h * 128:(h + 1) * 128], rhs,
                            start=first, stop=last, skip_group_check=True)
                        nc.tensor.matmul(
                            Dn[0:1, r:512] if r else Dn,
                            ones_sb[:, 0:1], rhs,
                            start=first, stop=last, skip_group_check=True)
                        if last:
                            rec = prec.tile([1, 512], f32, name="rec")
                            nc.vector.reciprocal(rec, Dn[0:1, :])
                            bsb = pbsb.tile([128, 512], f32, name="bsb")
                            nc.gpsimd.partition_broadcast(bsb, rec, 128)
                            nc.vector.tensor_mul(
                                att_sb[h][:, s * 512:(s + 1) * 512], A, bsb)

                    DEPTH = 4
                    pending = deque()
                    for h in range(NHL):
                        for s in range(NST):
                            seq = sched[s]
                            A = psa.tile([128, 512], f32, name="A")
                            Dn = psd.tile([1, 512], f32, name="Dn")
                            ch = (h, s, seq, A, Dn, [None] * len(seq))
                            for idx in range(len(seq)):
                                emit_scores(ch, idx)
                                pending.append((ch, idx))
                                if len(pending) >= DEPTH:
                                    emit_avdn(*pending.popleft())
                    while pending:
                        emit_avdn(*pending.popleft())

                # ---------------- Phase 3: output projection ---------------
                with tc.tile_pool(name="ps3", bufs=4, space="PSUM") as ps3p:
                    for c in range(8):
                        wt = w3_tiles[c]
                        if c + 1 < 8:
                            w3_tiles[c + 1] = load_w3(c + 1)
                        for m in range(NKT):
                            ps = ps3p.tile([128, 512], f32, name="ps3")
                            for k in range(NHL):
                                nc.tensor.matmul(
                                    ps, att_sb[k][:, m * 128:(m + 1) * 128],
                                    wt[:, k, :],
                                    start=(k == 0), stop=(k == NHL - 1))
                            o3 = po3.tile([128, 512], bf16, name="o3")
                            nc.scalar.activation(o3, ps, COPY)
                            nc.gpsimd.dma_start(
                                out=out_d[m * 128:(m + 1) * 128,
                                          c * 512:(c + 1) * 512],
                                in_=o3)

    nc.compile()
    return nc


def _host_prep(x, wq, wk, wv, wo, freqs_cos, freqs_sin, mask):
    """Build per-core input maps + mask block classes."""
    import ml_dtypes
    bf16 = ml_dtypes.bfloat16

    x = np.asarray(x, np.float32)
    wq = np.asarray(wq, np.float32)
    wk = np.asarray(wk, np.float32)
    wv = np.asarray(wv, np.float32)
    wo = np.asarray(wo, np.float32)
    mask2 = np.asarray(mask, np.float32).reshape(S, S)

    # de-interleave each head's features: evens then odds (consistent q/k)
    perm = np.concatenate(
        [hl * 128 + np.concatenate([np.arange(0, 128, 2), np.arange(1, 128, 2)])
         for hl in range(NHL)])
    cosT = np.asarray(freqs_cos, np.float32).T    # [64, S]
    sinT = np.asarray(freqs_sin, np.float32).T
    cos2 = np.ascontiguousarray(np.vstack([cosT, cosT]))
    sin2 = np.ascontiguousarray(np.vstack([-sinT, sinT]))

    # classify mask blocks on the transposed mask (rows = tk, cols = tq)
    maskt = mask2.T
    p_idx = np.arange(128)[:, None]
    f_idx = np.arange(512)[None, :]
    classes = [[0] * NST for _ in range(NKT)]
    x3_blocks = []
    for j in range(NKT):
        for s in range(NST):
            blk = maskt[j * 128:(j + 1) * 128, s * 512:(s + 1) * 512]
            r = j * 128 - s * 512
            if (blk <= NEG_THRESH).all():
                classes[j][s] = 0
            elif (blk == 0.0).all():
                classes[j][s] = 1
            else:
                ok = False
                if 0 <= r < 512:
                    causal_zero = (f_idx >= p_idx + r)
                    ok = ((blk == 0.0) == causal_zero).all() and \
                         (blk[~causal_zero] <= NEG_THRESH).all()
                if ok:
                    classes[j][s] = 2
                else:
                    classes[j][s] = 3
                    x3_blocks.append(blk)
    n3 = len(x3_blocks)
    if n3:
        x3 = np.ascontiguousarray(
            np.concatenate(x3_blocks, axis=1)).astype(np.float32)
    else:
        x3 = np.zeros((128, 512), np.float32)

    m0 = np.where(f_idx >= p_idx, 0.0, NEG).astype(np.float32)

    scale = np.float32(1.0 / np.sqrt(HD))

    def tile_qk(w_c):
        # wg[i][p][k][f] = w_c[i*128+f, k*128+p]
        wt = w_c.reshape(NHL, 128, KT, 128)          # [i, f, k, p]
        wt = wt.transpose(0, 3, 2, 1)                # [i, p, k, f]
        return np.ascontiguousarray(wt.reshape(NHL * 128, KT * 128))

    def tile_wv(wv_c):
        # wvg[fc*4+g][p][k][f] = wv_c[fc*512+f, (g*8+k)*128+p]
        wt = wv_c.reshape(2, 512, 4, 8, 128)         # [fc, f, g, k, p]
        wt = wt.transpose(0, 2, 4, 3, 1)             # [fc, g, p, k, f]
        return np.ascontiguousarray(wt.reshape(8 * 128, 8 * 512))

    def tile_wo(wo_c):
        # wog[c][p][k][f] = wo_c[c*512+f, k*128+p]   (wo_c = wo[:, sl])
        wt = wo_c.reshape(8, 512, NHL, 128)          # [c, f, k, p]
        wt = wt.transpose(0, 3, 2, 1)                # [c, p, k, f]
        return np.ascontiguousarray(wt.reshape(8 * 128, NHL * 512))

    def tile_x(xb):
        # xg[st][p][k][f] = xb[st*512+f, k*128+p]     (xb = x[b] [S, D])
        xt = xb.reshape(NST, 512, KT, 128)           # [st, f, k, p]
        xt = xt.transpose(0, 3, 2, 1)                # [st, p, k, f]
        return np.ascontiguousarray(xt.reshape(NST * 128, KT * 512))

    xgs = [tile_x(x[b]).astype(bf16) for b in range(2)]
    cos2_b = cos2.astype(bf16)
    sin2_b = sin2.astype(bf16)
    m0_b = m0.astype(bf16)
    ones_b = np.ones((128, 8), bf16)
    in_maps = []
    for core in range(8):
        b, tp = core // 4, core % 4
        sl = slice(tp * FSH, (tp + 1) * FSH)
        wq_c = wq[sl][perm] * scale
        wk_c = wk[sl][perm]
        in_maps.append({
            "xg": xgs[b],
            "wqg": tile_qk(wq_c).astype(bf16),
            "wkg": tile_qk(wk_c).astype(bf16),
            "wvg": tile_wv(wv[sl]).astype(bf16),
            "wog": tile_wo(wo[:, sl]).astype(bf16),
            "cos2": cos2_b,
            "sin2": sin2_b,
            "m0": m0_b,
            "ones8": ones_b,
            "x3": x3,
        })
    return in_maps, classes, n3


def kernel(x, wq, wk, wv, wo, freqs_cos, freqs_sin, mask, start_pos=0,
           _trace=False):
    from concourse import bass_utils
    in_maps, classes, n3 = _host_prep(x, wq, wk, wv, wo, freqs_cos, freqs_sin,
                                      mask)
    key = (str(classes), n3)
    if key not in _cache:
        _cache[key] = _build(classes, n3)
    nc = _cache[key]
    res = bass_utils.run_bass_kernel_spmd(nc, in_maps, core_ids=list(range(8)),
                                          trace=_trace)
    out = np.zeros((2, S, D), np.float32)
    for core in range(8):
        out[core // 4] += np.asarray(res.results[core]["out"],
                                     dtype=np.float32)
    kernel.last_result = res
    return out


if __name__ == "__main__":
    # compile-only smoke test with causal classes
    classes = [[0] * NST for _ in range(NKT)]
    for j in range(NKT):
        for s in range(NST):
            r = j * 128 - s * 512
            if r >= 512:
                classes[j][s] = 0
            elif r <= -128:
                classes[j][s] = 1
            else:
                classes[j][s] = 2
    import time
    t0 = time.time()
    nc = _build(classes, 0)
    print(f"build+bacc-compile: {time.time()-t0:.1f}s")
    try:
        from concourse.timeline_sim import TimelineSim
        est = TimelineSim(nc, trace=False).simulate()
        print(f"TimelineSim per-core exec estimate: {est:.0f} ns")
    except Exception as e:
        print("TimelineSim unavailable:", e)
    if len(sys.argv) > 1 and sys.argv[1] == "neff":
        import tempfile
        from concourse import bass_utils
        t0 = time.time()
        with tempfile.TemporaryDirectory() as td:
            bass_utils.compile_bass_kernel(nc, td)
            print(f"walrus: {time.time()-t0:.1f}s COMPILED OK")
